# revision 17
# baseline (speedup 1.0000x reference)
"""Trainium2 Bass kernel for nn_Decoder sparse-attention decode step.

Reference computation (n=200000, d=128):
    f = concat([x, X[s], X[p]]); q = f @ Wq
    u = (X @ Wk) @ q / sqrt(d)
    u_ = softmax(u + mask)          # mask: 1 everywhere, 0 at visited
    out = (u_ @ (X @ Wv)) @ Wo

Algebraic restructure (exact in exact arithmetic):
    w   = Wk @ q / sqrt(d)                      # [d]  (host, O(d^2))
    u   = X @ w                                 # one streaming pass over X
    p_r = exp(u_r) * fsel_r                     # fsel: 1 / e^-1 visited / 0 pad
    acc = sum_r p_r X_r ; S = sum_r p_r
    out = (acc @ (Wv @ Wo)) / S                 # Wv@Wo applied on host

Sharding: X rows split across 8 NeuronCores (25000 rows each, zero-padded
to 25088 = 196*128).  Each core ships partial (acc, S); the host applies
Wv@Wo, sums the 8 partials and divides (exp never overflows: |u| < ~4).

Per-core schedule (cost-model-driven):
  - X streamed as bf16 (halves the 360 GB/s DMA floor to ~17.8us)
  - dot u_tile = sum_f X_tile * w split three ways to balance devices:
      DVE scalar_tensor_tensor+accum (~195ns/tile)
      GpSimd scalar_tensor_tensor    (~274ns/tile)
      PE matmul on tiles transposed in-SBUF by the DMA xbar
        (dma_start_transpose, ~112ns/tile marginal on the DMA device;
         whole chunks at a time so the HWDGE setup cost amortizes)
  - exp on ACT per chunk; p = exp(u)*fsel on DVE (handles visited+pad)
  - acc += X_tile^T p_col on PE (4ns/matmul, free)
  - epilogue: S = ones^T scol on PE, single small output DMA on SP
"""

import os
import sys

import numpy as np
import ml_dtypes

_REPO = "/opt/trn_rl_repo"
if _REPO not in sys.path:
    sys.path.insert(0, _REPO)

import concourse.bacc as bacc
import concourse.bass_utils as bass_utils
import concourse.mybir as mybir
from concourse import tile

P = 128                    # hidden dim / partition count
NCORES = 8
NROWS = 25000              # rows per core
RP = 25088                 # padded rows per core (= 196 * 128)
T = RP // P                # 196 tiles of 128 rows
ONE_M_EINV = 0.6321205588285577  # 1 - exp(-1); kept for test harness
EINV = float(np.exp(-1.0))

F32 = mybir.dt.float32
BF16 = mybir.dt.bfloat16
BF = ml_dtypes.bfloat16


def _chunk_plan():
    """(size, kind) per chunk; kind 'n' = DVE/Pool dots, 'p' = PE via
    DRAM-sourced xbar-transposed tiles.  Only 8 HWDGE semaphore lanes exist,
    so the whole program must stay near <=11 DMA instructions or DMA issue
    serializes against lagging consumers.  Chunk 0 rides in the consts DMA."""
    pesz = int(os.environ.get("KPESZ", "15"))
    npe = int(os.environ.get("KNPE", "2"))
    plan = [(4, "n"), (24, "n")]
    if npe >= 1:
        plan.append((pesz, "p"))
    plan.append((30, "n"))
    if npe >= 2:
        plan.append((pesz, "p"))
    rem = T - sum(s for s, _ in plan) - 18
    while rem > 0:
        s = min(30, rem)
        plan.append((s, "n"))
        rem -= s
    plan += [(12, "n"), (6, "n")]
    assert sum(s for s, _ in plan) == T, sum(s for s, _ in plan)
    return plan

CHP = _chunk_plan()
NCHUNK = len(CHP)
NPE_TILES = sum(s for s, k in CHP if k == "p")
POOL_NUM = int(os.environ.get("KPOOLN", "5"))   # pool tiles per 12 (non-PE)
POOL_DEN = 12

# cpack columns (all bf16): [0:128) wb broadcast | [128:129) wcol | [129:325) fsel
# then chunk 0's X tiles ride along in the same DMA
CC = 325
CH0 = CHP[0][0]
CCX = CC + CH0 * P

_CACHE = {}


def _build_program():
    if "nc" in _CACHE:
        return _CACHE["nc"]

    nc = bacc.Bacc(
        "TRN2",
        target_bir_lowering=False,
        debug=False,
        enable_asserts=False,
        num_devices=NCORES,
    )

    xs_d = nc.dram_tensor("xs", [RP, P], BF16, kind="ExternalInput")
    cp_d = nc.dram_tensor("cpack", [P, CCX], BF16, kind="ExternalInput")
    # col 0: acc partial; [0,1]: S partial  (single output DMA)
    o_d = nc.dram_tensor("o_part", [P, 2], F32, kind="ExternalOutput")

    # X rows laid out partition-major: partition p holds rows [T*p, T*(p+1))
    xs_re = xs_d.ap().rearrange("(p t) f -> p t f", p=P)

    choff = []
    _o = 0
    for s, _k in CHP:
        choff.append(_o)
        _o += s

    with tile.TileContext(nc) as tc:
        with (
            tc.tile_pool(name="const", bufs=1) as cpool,
            tc.tile_pool(name="xpool", bufs=1) as xpool,
            tc.tile_pool(name="work", bufs=1) as wpool,
            tc.tile_pool(name="scr", bufs=4) as spool,
            tc.tile_pool(name="scrg", bufs=4) as gpool,
            tc.tile_pool(name="trp", bufs=2) as trpool,
            tc.tile_pool(name="ppool", bufs=1, space="PSUM") as ppool,
        ):
            # ---- constants + chunk 0: one packed DMA, issued first on SP ----
            cp_sb = cpool.tile([P, CCX], BF16, tag="cpack")
            nc.sync.dma_start(cp_sb[:], cp_d.ap())
            wb_sb = cp_sb[:, 0:128]       # w broadcast along partitions
            wcol_sb = cp_sb[:, 128:129]   # w on partitions
            fsel_sb = cp_sb[:, 129:325]   # [p, t] select factor
            x0_view = cp_sb[:, CC:CCX].rearrange("p (t f) -> p t f", t=CH0)

            ones_col = cpool.tile([P, 1], F32, tag="ones_col")
            nc.vector.memset(ones_col[:], 1.0)
            opk_sb = wpool.tile([P, 2], F32, tag="opk")
            nc.vector.memset(opk_sb[:], 0.0)

            # ---- X chunks: all DMAs issued up front on SP, HWDGE-paced.
            # PE chunks additionally get a DRAM-sourced xbar-transposed copy
            # (no dependency on the row-major chunk DMA, so SP never stalls).
            x_sb = []
            xT_sb = {}
            for c, (tc_n, kind) in enumerate(CHP):
                if c == 0:
                    x_sb.append(x0_view)
                    continue
                xt = xpool.tile([P, tc_n, P], BF16, tag=f"x{c}", name=f"x{c}")
                nc.sync.dma_start(xt[:], xs_re[:, choff[c]: choff[c] + tc_n, :])
                x_sb.append(xt)
                if kind == "p":
                    xT = xpool.tile([P, tc_n, P], BF16, tag=f"xT{c}",
                                    name=f"xT{c}")
                    nc.sync.dma_start_transpose(
                        xT[:], xs_re[:, choff[c]: choff[c] + tc_n, :])
                    xT_sb[c] = xT

            u_sb = cpool.tile([P, T], F32, tag="u")
            u_ps = ppool.tile([P, max(NPE_TILES, 1)], F32, tag="u_ps")
            scol_sb = wpool.tile([P, NCHUNK], F32, tag="scol")
            p_sb = []
            acc_ps = ppool.tile([P, 1], F32, tag="acc_ps")
            pe_col = [0]

            def emit_dots(c):
                tc_n, kind = CHP[c]
                lo = choff[c]
                if kind == "p":
                    return
                for i in range(tc_n):
                    j = lo + i
                    on_pool = (j * POOL_NUM) % POOL_DEN < POOL_NUM
                    pool = gpool if on_pool else spool
                    eng = nc.gpsimd if on_pool else nc.vector
                    scr = pool.tile([P, P], BF16,
                                    tag=("sg" if on_pool else "sv"), name="scr")
                    eng.scalar_tensor_tensor(
                        out=scr[:],
                        in0=x_sb[c][:, i, :],
                        scalar=1.0,
                        in1=wb_sb[:],
                        op0=mybir.AluOpType.mult,
                        op1=mybir.AluOpType.mult,
                        accum_out=u_sb[:, j: j + 1],
                    )

            def emit_tail(c):
                """(PE u-dots), exp, fsel-mult (+S accum), acc matmuls."""
                tc_n, kind = CHP[c]
                lo = choff[c]
                if kind == "p":
                    k0 = pe_col[0]
                    for i in range(tc_n):
                        nc.tensor.matmul(
                            u_ps[:, k0 + i: k0 + i + 1],
                            xT_sb[c][:, i, :],
                            wcol_sb[:],
                            start=True,
                            stop=True,
                            skip_group_check=True,
                        )
                    pe_col[0] = k0 + tc_n
                    nc.scalar.copy(u_sb[:, lo: lo + tc_n],
                                   u_ps[:, k0: k0 + tc_n])
                et = spool.tile([P, tc_n], F32, tag="et", name=f"e{c}")
                nc.scalar.activation(
                    et[:], u_sb[:, lo: lo + tc_n],
                    mybir.ActivationFunctionType.Exp,
                )
                pt = wpool.tile([P, tc_n], BF16, tag=f"p{c}", name=f"p{c}")
                p_sb.append(pt)
                nc.vector.scalar_tensor_tensor(
                    out=pt[:],
                    in0=et[:],
                    scalar=1.0,
                    in1=fsel_sb[:, lo: lo + tc_n],
                    op0=mybir.AluOpType.mult,
                    op1=mybir.AluOpType.mult,
                    accum_out=scol_sb[:, c: c + 1],
                )
                for i in range(tc_n):
                    j = lo + i
                    nc.tensor.matmul(
                        acc_ps[:],
                        x_sb[c][:, i, :],
                        pt[:, i: i + 1],
                        start=(j == 0),
                        stop=(j == T - 1),
                        skip_group_check=True,
                    )

            # lag the exp/fsel/acc of chunk c until after chunk c+1's dots so
            # in-order engines never head-of-line block on cross-engine deps
            LAG = int(os.environ.get("KLAG", "1"))
            for c in range(NCHUNK):
                emit_dots(c)
                if c >= LAG:
                    emit_tail(c - LAG)
            for c in range(NCHUNK - LAG, NCHUNK):
                emit_tail(c)

            # ---- epilogue: ship (acc, S); host applies Wv@Wo ----
            smain_col = wpool.tile([P, 1], F32, tag="smain")
            nc.vector.tensor_reduce(
                smain_col[:], scol_sb[:], mybir.AxisListType.X,
                mybir.AluOpType.add,
            )
            s_ps = ppool.tile([1, 1], F32, tag="s_ps")
            nc.tensor.matmul(s_ps[:], smain_col[:], ones_col[:],
                             skip_group_check=True)

            nc.scalar.copy(opk_sb[:, 0:1], acc_ps[:])
            nc.scalar.copy(opk_sb[0:1, 1:2], s_ps[:])
            nc.sync.dma_start(o_d.ap(), opk_sb[:])

    nc.compile()
    _CACHE["nc"] = nc
    return nc


def make_in_maps(X, x, Wq, Wk, Wv, Wo, nodes_visited, starting_node,
                 previous_node):
    X = np.asarray(X, dtype=np.float32)
    x = np.asarray(x, dtype=np.float32)
    Wq = np.asarray(Wq, dtype=np.float64)
    Wk = np.asarray(Wk, dtype=np.float64)
    vis = np.unique(np.asarray(nodes_visited).astype(np.int64))

    # host prologue: w = Wk @ (f @ Wq) / sqrt(d)
    f = np.concatenate([x, X[int(starting_node)], X[int(previous_node)]])
    q = f.astype(np.float64) @ Wq
    w = (Wk @ q) / np.sqrt(np.float64(P))

    Xb = X.astype(BF)

    in_maps = []
    for c in range(NCORES):
        lo, hi = c * NROWS, (c + 1) * NROWS
        xs = np.zeros((RP, P), BF)
        xs[:NROWS] = Xb[lo:hi]
        fsel = np.ones(RP, np.float32)
        sel = vis[(vis >= lo) & (vis < hi)] - lo
        fsel[sel] = EINV
        fsel[NROWS:] = 0.0
        cpack = np.zeros((P, CCX), BF)
        cpack[:, 0:128] = np.broadcast_to(w.astype(BF), (P, P))
        cpack[:, 128] = w.astype(BF)
        cpack[:, 129:325] = fsel.reshape(P, T).astype(BF)
        cpack[:, CC:] = xs.reshape(P, T, P)[:, :CH0, :].reshape(P, CH0 * P)
        in_maps.append({"xs": xs, "cpack": cpack})
    return in_maps


def combine(results, Wv=None, Wo=None):
    acc = np.zeros(P, np.float64)
    S = 0.0
    for r in results:
        acc += r["o_part"][:, 0].astype(np.float64)
        S += float(r["o_part"][0, 1])
    o = acc @ (np.asarray(Wv, np.float64) @ np.asarray(Wo, np.float64))
    return (o / S).astype(np.float32)


def kernel(X, x, Wq, Wk, Wv, Wo, nodes_visited, starting_node, previous_node,
           _trace=False):
    nc = _build_program()
    in_maps = make_in_maps(
        X, x, Wq, Wk, Wv, Wo, nodes_visited, starting_node, previous_node
    )
    res = bass_utils.run_bass_kernel_spmd(
        nc, in_maps, core_ids=list(range(NCORES)), trace=_trace
    )
    out = combine(res.results, Wv=Wv, Wo=Wo)
    if _trace:
        kernel.last_exec_time_ns = res.exec_time_ns
        kernel.last_profile = res.profile_json
    return out
